# revision 10
# baseline (speedup 1.0000x reference)
"""SSIM(3x3 avg-pool) + L1 loss kernel for Trainium2, 8 NeuronCores.

loss = 0.85 * mean(clip((1 - ssim_map)/2, 0, 1)) + 0.15 * mean(|pred - target|)

Full inputs pred/target: (16, 1, 1024, 1024) f32. Data-parallel: 2 images
per core; each core returns per-partition partial sums [128, 2]
(col 0 = sum of clipped ssim loss map, col 1 = sum |pred-target|);
host combines and applies the means / alpha-beta weights.

Math per core (H=1024 images, stacked rows):
  u = p + t, v = p - t
  box(x) = 3x3 zero-padded box sum / 9 (separable)
  X = box(p), Y = box(t), G = box(u^2), H = box(v^2)
  mu_x*mu_y       = X*Y
  sigma_xy - ...  : m2x+m2y = (G+H)/2, mxy = (G-H)/4
  n1 = 2XY + C1, n2 = (G-H)/2 - 2XY + C2
  d1 = X^2+Y^2 + C1, d2 = (G+H)/2 - (X^2+Y^2) + C2
  ssim_map = n1*n2/(d1*d2);  contrib = clip(0.5 - 0.5*ssim_map, 0, 1)

Horizontal 3-tap runs on DVE via shifted adds over zero-padded columns
(inputs are passed pre-padded to width W+2 with zero edge columns).
Vertical 3-tap runs on the TensorEngine as a banded [rows_in, rows_out]
matmul with 1/9 entries, which also handles the row-halo via overlapped
126-row output blocks fed from 128-row input stripes.
"""

import sys

import numpy as np

sys.path.insert(0, "/opt/trn_rl_repo")

ALPHA = 0.85
BETA = 0.15
C1 = 0.01 ** 2
C2 = 0.03 ** 2

N_CORES = 8
IMG_H = 1024
IMG_W = 1024
N_IMG_PER_CORE = 2
BLK = 126          # output rows per vertical-matmul block
MAXW_PSUM = 512    # fp32 columns per PSUM bank


def _blocks(H):
    """Vertical block decomposition: list of (r0, n_out, rs, nr)."""
    out = []
    b = 0
    while b * BLK < H:
        r0 = b * BLK
        n_out = min(BLK, H - r0)
        rs = max(r0 - 1, 0)
        re = min(r0 + n_out, H - 1)
        out.append((r0, n_out, rs, re - rs + 1))
        b += 1
    return out


def make_bmats(H):
    """Banded vertical-sum matrices (entries 1/9), padded into [nblk,128,BLK]."""
    blocks = _blocks(H)
    bm = np.zeros((len(blocks), 128, BLK), dtype=np.float32)
    ninth = np.float32(1.0) / np.float32(9.0)
    for i, (r0, n_out, rs, nr) in enumerate(blocks):
        for k in range(nr):
            for j in range(n_out):
                if abs((rs + k) - (r0 + j)) <= 1:
                    bm[i, k, j] = ninth
    return bm


def build_program(n_img, H, W):
    import concourse.bacc as bacc
    import concourse.tile as tile
    from concourse import mybir

    f32 = mybir.dt.float32
    Alu = mybir.AluOpType
    Act = mybir.ActivationFunctionType

    blocks = _blocks(H)
    nblk = len(blocks)
    Wp = W + 2  # padded width
    n_chunks = (W + MAXW_PSUM - 1) // MAXW_PSUM

    import os
    debug_dump = bool(int(os.environ.get("KERNEL_DEBUG_DUMP", "0")))

    nc = bacc.Bacc("TRN2", target_bir_lowering=False, debug=False)

    pred_d = nc.dram_tensor("pred", [n_img * H, Wp], f32, kind="ExternalInput").ap()
    targ_d = nc.dram_tensor("target", [n_img * H, Wp], f32, kind="ExternalInput").ap()
    bm_d = nc.dram_tensor("bmats", [nblk, 128, BLK], f32, kind="ExternalInput").ap()
    acc_d = nc.dram_tensor("acc_out", [128, 2], f32, kind="ExternalOutput").ap()
    dbg_d = {}
    if debug_dump:
        for dn in ("X", "Y", "G", "Hh", "num", "den", "mp", "yc", "h3p"):
            dbg_d[dn] = nc.dram_tensor(
                f"dbg_{dn}", [128, MAXW_PSUM], f32, kind="ExternalOutput"
            ).ap()

    with tile.TileContext(nc) as tc:
        with (
            tc.tile_pool(name="consts", bufs=1) as cpool,
            tc.tile_pool(name="io", bufs=3) as iopool,
            tc.tile_pool(name="fields", bufs=2) as fpool,
            tc.tile_pool(name="hsum", bufs=2) as hpool,
            tc.tile_pool(name="post", bufs=2) as ppool,
            tc.tile_pool(name="psum", bufs=2, space="PSUM") as psumpool,
        ):
            # persistent accumulator + B matrices
            acc = cpool.tile([128, 2], f32, tag="acc")
            nc.vector.memset(acc[:, :], 0.0)
            bmats = []
            for i, (r0, n_out, rs, nr) in enumerate(blocks):
                bt = cpool.tile([128, BLK], f32, tag=f"bmat{i}")
                nc.sync.dma_start(out=bt[0:nr, 0:n_out], in_=bm_d[i, 0:nr, 0:n_out])
                bmats.append(bt)

            for img in range(n_img):
                base = img * H
                for bi, (r0, n_out, rs, nr) in enumerate(blocks):
                    # local row range of the (non-halo) output rows in stripe
                    lo = r0 - rs
                    hi = lo + n_out

                    p = iopool.tile([128, Wp], f32, tag="p")
                    t = iopool.tile([128, Wp], f32, tag="t")
                    nc.sync.dma_start(out=p[0:nr, :], in_=pred_d[base + rs: base + rs + nr, :])
                    nc.sync.dma_start(out=t[0:nr, :], in_=targ_d[base + rs: base + rs + nr, :])

                    u = fpool.tile([128, Wp], f32, tag="u")
                    v = fpool.tile([128, Wp], f32, tag="v")
                    nc.vector.tensor_add(u[0:nr, :], p[0:nr, :], t[0:nr, :])
                    nc.vector.tensor_sub(v[0:nr, :], p[0:nr, :], t[0:nr, :])

                    # L1 partial: |v| over a disjoint cover of rows. Stripe b
                    # starts at rs_b; local rows [0:k] with k = rs_{b+1}-rs_b
                    # (nr for the last stripe) tile all rows exactly once and
                    # keep the partition offset at 0 (pad cols are 0).
                    if bi + 1 < len(blocks):
                        k_l1 = blocks[bi + 1][2] - rs
                    else:
                        k_l1 = nr
                    l1part = ppool.tile([128, 1], f32, tag="l1part")
                    nc.scalar.activation(
                        v[0:k_l1, :], v[0:k_l1, :], Act.Abs,
                        accum_out=l1part[0:k_l1, :],
                    )
                    u2 = fpool.tile([128, Wp], f32, tag="u2")
                    v2 = fpool.tile([128, Wp], f32, tag="v2")
                    nc.scalar.activation(u2[0:nr, :], u[0:nr, :], Act.Square)
                    nc.scalar.activation(v2[0:nr, :], v[0:nr, :], Act.Square)

                    # horizontal 3-tap sums (zero-padded edges)
                    hs = {}
                    for name, src in (("p", p), ("t", t), ("u2", u2), ("v2", v2)):
                        g = hpool.tile([128, W + 1], f32, tag="g")
                        nc.vector.tensor_add(
                            g[0:nr, :], src[0:nr, 0:W + 1], src[0:nr, 1:W + 2]
                        )
                        h3 = hpool.tile([128, W], f32, tag=f"h3{name}")
                        nc.vector.tensor_add(
                            h3[0:nr, :], g[0:nr, 0:W], src[0:nr, 2:W + 2]
                        )
                        hs[name] = h3

                    bmat = bmats[bi]
                    for ci in range(n_chunks):
                        c0 = ci * MAXW_PSUM
                        cw = min(MAXW_PSUM, W - c0)
                        # vertical 3-tap via banded matmul -> pooled values
                        ps = {}
                        for name in ("p", "t", "u2", "v2"):
                            pt = psumpool.tile([128, MAXW_PSUM], f32, tag=f"ps{name}")
                            nc.tensor.matmul(
                                pt[0:n_out, 0:cw],
                                lhsT=bmat[0:nr, 0:n_out],
                                rhs=hs[name][0:nr, c0:c0 + cw],
                                start=True, stop=True,
                            )
                            ps[name] = pt
                        X, Y = ps["p"], ps["t"]
                        G, Hh = ps["u2"], ps["v2"]
                        ro = slice(0, n_out)
                        co = slice(0, cw)

                        def pt_(tag):
                            return ppool.tile([128, MAXW_PSUM], f32, tag=tag,
                                              name=tag)

                        # only one PSUM operand allowed per instruction:
                        # stage Y and Hh into SBUF first
                        Ysb = pt_("Ysb")
                        nc.scalar.copy(Ysb[ro, co], Y[ro, co])
                        Hsb = pt_("Hsb")
                        nc.scalar.copy(Hsb[ro, co], Hh[ro, co])
                        A = pt_("A")
                        nc.vector.tensor_mul(A[ro, co], X[ro, co], Ysb[ro, co])
                        sx = pt_("sx")
                        sy = pt_("sy")
                        nc.scalar.activation(sx[ro, co], X[ro, co], Act.Square)
                        nc.scalar.activation(sy[ro, co], Ysb[ro, co], Act.Square)
                        S = pt_("S")
                        nc.vector.tensor_add(S[ro, co], sx[ro, co], sy[ro, co])
                        Dd = pt_("Dd")
                        nc.vector.tensor_sub(Dd[ro, co], G[ro, co], Hsb[ro, co])
                        M = pt_("M")
                        nc.vector.tensor_add(M[ro, co], G[ro, co], Hsb[ro, co])
                        # n1 = 2A + C1 ; A2 = 2A - C2 ; n2 = 0.5*Dd - A2
                        n1 = pt_("n1")
                        nc.scalar.activation(n1[ro, co], A[ro, co], Act.Copy,
                                             bias=float(C1), scale=2.0)
                        A2 = pt_("A2")
                        nc.scalar.activation(A2[ro, co], A[ro, co], Act.Copy,
                                             bias=float(-C2), scale=2.0)
                        n2 = pt_("n2")
                        nc.vector.scalar_tensor_tensor(
                            n2[ro, co], Dd[ro, co], 0.5, A2[ro, co],
                            op0=Alu.mult, op1=Alu.subtract,
                        )
                        # d1 = S + C1 ; Sc = S - C2 ; d2 = 0.5*M - Sc
                        d1 = pt_("d1")
                        nc.scalar.activation(d1[ro, co], S[ro, co], Act.Copy,
                                             bias=float(C1), scale=1.0)
                        Sc = pt_("Sc")
                        nc.scalar.activation(Sc[ro, co], S[ro, co], Act.Copy,
                                             bias=float(-C2), scale=1.0)
                        d2 = pt_("d2")
                        nc.vector.scalar_tensor_tensor(
                            d2[ro, co], M[ro, co], 0.5, Sc[ro, co],
                            op0=Alu.mult, op1=Alu.subtract,
                        )
                        num = pt_("num")
                        nc.vector.tensor_mul(num[ro, co], n1[ro, co], n2[ro, co])
                        den = pt_("den")
                        nc.vector.tensor_mul(den[ro, co], d1[ro, co], d2[ro, co])
                        rcp = pt_("rcp")
                        scr = pt_("scr")
                        nc.vector.reciprocal_approx_accurate(
                            rcp[ro, co], den[ro, co], scr[ro, co]
                        )
                        mp = pt_("mp")
                        nc.vector.tensor_mul(mp[ro, co], num[ro, co], rcp[ro, co])
                        # clip((1-m)/2, 0, 1) = 0.5 - 0.5*clamp(m, -1, 1)
                        yv = pt_("yv")
                        nc.vector.tensor_scalar(
                            yv[ro, co], mp[ro, co], -1.0, 1.0,
                            op0=Alu.max, op1=Alu.min,
                        )
                        yc = pt_("yc")
                        spart = ppool.tile([128, 1], f32, tag="spart")
                        nc.scalar.activation(yc[ro, co], yv[ro, co], Act.Copy,
                                             bias=0.5, scale=-0.5,
                                             accum_out=spart[ro, :])
                        nc.vector.tensor_add(
                            acc[0:n_out, 0:1], acc[0:n_out, 0:1], spart[ro, :]
                        )
                        if debug_dump and img == 0 and bi == 0 and ci == 0:
                            for dn, src in (
                                ("X", X), ("Y", Y), ("G", G), ("Hh", Hh),
                                ("num", num), ("den", den), ("mp", mp),
                                ("yc", yc), ("h3p", None),
                            ):
                                if dn == "h3p":
                                    nc.sync.dma_start(
                                        out=dbg_d[dn][0:nr, co],
                                        in_=hs["p"][0:nr, c0:c0 + cw])
                                else:
                                    stg = ppool.tile([128, MAXW_PSUM], f32,
                                                     tag="dbgstg", name="dbgstg")
                                    nc.scalar.copy(stg[ro, co], src[ro, co])
                                    nc.sync.dma_start(
                                        out=dbg_d[dn][ro, co], in_=stg[ro, co])
                    nc.vector.tensor_add(
                        acc[0:k_l1, 1:2], acc[0:k_l1, 1:2], l1part[0:k_l1, :]
                    )

            nc.sync.dma_start(out=acc_d[:, :], in_=acc[:, :])

    nc.compile()
    return nc


_CACHE = {}


def _get_program(n_img, H, W):
    key = (n_img, H, W)
    if key not in _CACHE:
        _CACHE[key] = build_program(n_img, H, W)
    return _CACHE[key]


def _pad_cols(x):
    """Add one zero column on each side of the last dim."""
    r, w = x.shape
    out = np.zeros((r, w + 2), dtype=np.float32)
    out[:, 1:w + 1] = x
    return out


LAST_RESULTS = None


def kernel(pred, target):
    import os

    from concourse.bass_utils import run_bass_kernel_spmd

    global LAST_RESULTS

    pred = np.asarray(pred, dtype=np.float32).reshape(16, IMG_H, IMG_W)
    target = np.asarray(target, dtype=np.float32).reshape(16, IMG_H, IMG_W)

    nc = _get_program(N_IMG_PER_CORE, IMG_H, IMG_W)
    bm = make_bmats(IMG_H)

    in_maps = []
    for c in range(N_CORES):
        sl = slice(c * N_IMG_PER_CORE, (c + 1) * N_IMG_PER_CORE)
        p = pred[sl].reshape(N_IMG_PER_CORE * IMG_H, IMG_W)
        t = target[sl].reshape(N_IMG_PER_CORE * IMG_H, IMG_W)
        in_maps.append({
            "pred": _pad_cols(p),
            "target": _pad_cols(t),
            "bmats": bm,
        })

    trace = bool(int(os.environ.get("KERNEL_TRACE", "0")))
    res = run_bass_kernel_spmd(nc, in_maps, list(range(N_CORES)), trace=trace)
    LAST_RESULTS = res
    ssim_sum = 0.0
    l1_sum = 0.0
    for r in res.results:
        acc = r["acc_out"]
        ssim_sum += float(acc[:, 0].sum(dtype=np.float64))
        l1_sum += float(acc[:, 1].sum(dtype=np.float64))
    n = 16.0 * IMG_H * IMG_W
    loss = ALPHA * (ssim_sum / n) + BETA * (l1_sum / n)
    return np.float32(loss)


# revision 19
# speedup vs baseline: 1.0980x; 1.0980x over previous
"""SSIM(3x3 avg-pool) + L1 loss kernel for Trainium2, 8 NeuronCores.

loss = 0.85 * mean(clip((1 - ssim_map)/2, 0, 1)) + 0.15 * mean(|pred - target|)

Full inputs pred/target: (16, 1, 1024, 1024) f32. Data-parallel: 2 images
per core; each core returns per-partition partial sums [128, 2]
(col 0 = sum of clipped ssim loss map, col 1 = sum |pred-target|);
host combines and applies the means / alpha-beta weights.

Math per core (H=1024 images, stacked rows):
  u = p + t, v = p - t
  box(x) = 3x3 zero-padded box sum / 9 (separable)
  X = box(p), Y = box(t), G = box(u^2), H = box(v^2)
  mu_x*mu_y       = X*Y
  sigma_xy - ...  : m2x+m2y = (G+H)/2, mxy = (G-H)/4
  n1 = 2XY + C1, n2 = (G-H)/2 - 2XY + C2
  d1 = X^2+Y^2 + C1, d2 = (G+H)/2 - (X^2+Y^2) + C2
  ssim_map = n1*n2/(d1*d2);  contrib = clip(0.5 - 0.5*ssim_map, 0, 1)

Horizontal 3-tap runs on DVE via shifted adds over zero-padded columns
(inputs are passed pre-padded to width W+2 with zero edge columns).
Vertical 3-tap runs on the TensorEngine as a banded [rows_in, rows_out]
matmul with 1/9 entries, which also handles the row-halo via overlapped
126-row output blocks fed from 128-row input stripes.
"""

import sys

import numpy as np

sys.path.insert(0, "/opt/trn_rl_repo")

ALPHA = 0.85
BETA = 0.15
C1 = 0.01 ** 2
C2 = 0.03 ** 2

N_CORES = 8
IMG_H = 1024
IMG_W = 1024
N_IMG_PER_CORE = 2
BLK = 126          # output rows per vertical-matmul block
MAXW_PSUM = 512    # fp32 columns per PSUM bank

# --- custom fused DVE ops (registered into concourse.dve_ops at import) --- #
_OP_SQSUM = None       # out = in0^2 + in1^2
_OP_SSIM_RAT = None    # out = (in0 + s0) * (in1*s1 - in0 + imm2)
_OP_SSIM_FINAL = None  # out = (s0 - clamp(in0*in1, s1, s0))*imm2; accum += out
_CUSTOM_OPS_OK = False


def _register_custom_ops():
    global _OP_SQSUM, _OP_SSIM_RAT, _OP_SSIM_FINAL, _CUSTOM_OPS_OK
    if _CUSTOM_OPS_OK:
        return
    import numpy as np
    from operator import add

    import concourse.dve_ops as dv
    from concourse.dve_spec import (
        C0, C1, C2, Spec, Src0, Src1, Zero, lower, maxx, minn, sq,
    )
    from concourse.dve_uop import DveOpSpec

    def _sqsum_ref(in0, in1, c0, c1, c2):
        return in0.astype(np.float32) ** 2 + in1.astype(np.float32) ** 2

    def _rat_ref(in0, in1, c0, c1, c2):
        a = in0.astype(np.float32)
        return (a + c0) * (in1.astype(np.float32) * c1 - a + c2)

    def _final_ref(in0, in1, c0, c1, c2):
        z = in0.astype(np.float32) * in1.astype(np.float32)
        b = (c0 - np.clip(z, c1, c0)) * c2
        b = b.astype(np.float32)
        return b, b.reshape(b.shape[0], -1).sum(axis=-1, keepdims=True)

    defs = [
        ("SSIM_SQSUM_ANT", Spec(body=sq(Src0) + sq(Src1), reference=_sqsum_ref)),
        ("SSIM_RAT_ANT", Spec(
            body=(Src0 + C0) * (Src1 * C1 - Src0 + C2), reference=_rat_ref)),
        ("SSIM_FINAL_ANT", Spec(
            body=(C0 - maxx(minn(Src0 * Src1, C0), C1)) * C2,
            accum=add, accum_init=Zero, reference=_final_ref)),
    ]
    made = {}
    for name, spec in defs:
        if name not in dv._SUB_OPCODE_FOR_NAME:
            stub = dv.DveOp(name, spec, subdim=False, uops_sha={})
            dv.OPS.append(stub)
            dv._SUB_OPCODE_FOR_NAME[name] = (
                dv._CUSTOM_DVE_ROW_BASE + len(dv.OPS) - 1
            )
            dv.CUSTOM_DVE_SPECS[name] = spec
        opcode = dv._SUB_OPCODE_FOR_NAME[name]
        shas = {}
        for ver in ("v3", "v4"):
            res = DveOpSpec(
                name=name, opcode=opcode, uops=lower(spec, ver=ver),
                rd1_en=dv.has_src1(spec),
            )
            shas[ver] = res.sha(ver)
        op = dv.DveOp(name, spec, subdim=False, uops_sha=shas)
        idx = next(i for i, o in enumerate(dv.OPS) if o.name == name)
        dv.OPS[idx] = op
        dv.CUSTOM_DVE_SPECS[name] = spec
        made[name] = op
    _OP_SQSUM = made["SSIM_SQSUM_ANT"]
    _OP_SSIM_RAT = made["SSIM_RAT_ANT"]
    _OP_SSIM_FINAL = made["SSIM_FINAL_ANT"]
    _CUSTOM_OPS_OK = True


def _blocks(H):
    """Vertical block decomposition: list of (r0, n_out, rs, nr)."""
    out = []
    b = 0
    while b * BLK < H:
        r0 = b * BLK
        n_out = min(BLK, H - r0)
        rs = max(r0 - 1, 0)
        re = min(r0 + n_out, H - 1)
        out.append((r0, n_out, rs, re - rs + 1))
        b += 1
    return out


def make_bmats(H):
    """Banded vertical-sum matrices (entries 1/9), padded into [nblk,128,BLK]."""
    blocks = _blocks(H)
    bm = np.zeros((len(blocks), 128, BLK), dtype=np.float32)
    ninth = np.float32(1.0) / np.float32(9.0)
    for i, (r0, n_out, rs, nr) in enumerate(blocks):
        for k in range(nr):
            for j in range(n_out):
                if abs((rs + k) - (r0 + j)) <= 1:
                    bm[i, k, j] = ninth
    return bm


def build_program(n_img, H, W, repeat=1, io_internal=False):
    """repeat>1 wraps the whole compute in a hardware loop (timing only;
    accumulator output becomes repeat x too large). io_internal makes
    pred/target Internal DRAM (garbage data, no host transfer) for
    timing-only builds."""
    import concourse.bacc as bacc
    import concourse.tile as tile
    from concourse import mybir

    f32 = mybir.dt.float32
    Alu = mybir.AluOpType
    Act = mybir.ActivationFunctionType

    blocks = _blocks(H)
    nblk = len(blocks)
    Wp = W + 2  # padded width
    n_chunks = (W + MAXW_PSUM - 1) // MAXW_PSUM

    _register_custom_ops()
    nc = bacc.Bacc("TRN2", target_bir_lowering=False, debug=False)

    io_kind = "Internal" if io_internal else "ExternalInput"
    pred_d = nc.dram_tensor("pred", [n_img * H, Wp], f32, kind=io_kind).ap()
    targ_d = nc.dram_tensor("target", [n_img * H, Wp], f32, kind=io_kind).ap()
    bm_d = nc.dram_tensor("bmats", [nblk, 128, BLK], f32, kind="ExternalInput").ap()
    acc_d = nc.dram_tensor("acc_out", [128, 2], f32, kind="ExternalOutput").ap()

    with tile.TileContext(nc) as tc:
        with (
            tc.tile_pool(name="consts", bufs=1) as cpool,
            tc.tile_pool(name="io", bufs=3) as iopool,
            tc.tile_pool(name="fields", bufs=2) as fpool,
            tc.tile_pool(name="hsum", bufs=2) as hpool,
            tc.tile_pool(name="post", bufs=2) as ppool,
            tc.tile_pool(name="psum", bufs=1, space="PSUM") as psumpool,
        ):
            # persistent accumulator + B matrices
            acc = cpool.tile([128, 2], f32, tag="acc")
            nc.vector.memset(acc[:, :], 0.0)
            if io_internal:
                # fill the internal input DRAM with a benign pattern so
                # timing isn't skewed by garbage (denormals/NaN)
                fill = cpool.tile([128, Wp], f32, tag="fill")
                nc.vector.memset(fill[:, :], 0.625)
                rows_total = n_img * H
                for r in range(0, rows_total, 128):
                    nrr = min(128, rows_total - r)
                    nc.sync.dma_start(out=pred_d[r:r + nrr, :], in_=fill[0:nrr, :])
                    nc.sync.dma_start(out=targ_d[r:r + nrr, :], in_=fill[0:nrr, :])
            bmats = []
            for i, (r0, n_out, rs, nr) in enumerate(blocks):
                bt = cpool.tile([128, BLK], f32, tag=f"bmat{i}")
                nc.sync.dma_start(out=bt[0:nr, 0:n_out], in_=bm_d[i, 0:nr, 0:n_out])
                bmats.append(bt)

            import contextlib
            rep_ctx = (
                tc.For_i(0, repeat, 1) if repeat > 1 else contextlib.nullcontext()
            )
            with rep_ctx:
              for img in range(n_img):
                base = img * H
                for bi, (r0, n_out, rs, nr) in enumerate(blocks):
                    # local row range of the (non-halo) output rows in stripe
                    lo = r0 - rs
                    hi = lo + n_out

                    p = iopool.tile([128, Wp], f32, tag="p")
                    t = iopool.tile([128, Wp], f32, tag="t")
                    nc.sync.dma_start(out=p[0:nr, :], in_=pred_d[base + rs: base + rs + nr, :])
                    nc.sync.dma_start(out=t[0:nr, :], in_=targ_d[base + rs: base + rs + nr, :])

                    u = fpool.tile([128, Wp], f32, tag="u")
                    v = fpool.tile([128, Wp], f32, tag="v")
                    nc.vector.tensor_add(u[0:nr, :], p[0:nr, :], t[0:nr, :])
                    nc.vector.tensor_sub(v[0:nr, :], p[0:nr, :], t[0:nr, :])

                    # L1 partial: |v| over a disjoint cover of rows. Stripe b
                    # starts at rs_b; local rows [0:k] with k = rs_{b+1}-rs_b
                    # (nr for the last stripe) tile all rows exactly once and
                    # keep the partition offset at 0 (pad cols are 0).
                    if bi + 1 < len(blocks):
                        k_l1 = blocks[bi + 1][2] - rs
                    else:
                        k_l1 = nr
                    l1part = ppool.tile([128, 1], f32, tag="l1part")
                    nc.scalar.activation(
                        v[0:k_l1, :], v[0:k_l1, :], Act.Abs,
                        accum_out=l1part[0:k_l1, :],
                    )
                    u2 = fpool.tile([128, Wp], f32, tag="u2")
                    v2 = fpool.tile([128, Wp], f32, tag="v2")
                    nc.scalar.activation(u2[0:nr, :], u[0:nr, :], Act.Square)
                    nc.scalar.activation(v2[0:nr, :], v[0:nr, :], Act.Square)

                    # horizontal 3-tap sums (zero-padded edges)
                    hs = {}
                    for name, src in (("p", p), ("t", t), ("u2", u2), ("v2", v2)):
                        g = hpool.tile([128, W + 1], f32, tag="g")
                        nc.vector.tensor_add(
                            g[0:nr, :], src[0:nr, 0:W + 1], src[0:nr, 1:W + 2]
                        )
                        h3 = hpool.tile([128, W], f32, tag=f"h3{name}")
                        nc.vector.tensor_add(
                            h3[0:nr, :], g[0:nr, 0:W], src[0:nr, 2:W + 2]
                        )
                        hs[name] = h3

                    bmat = bmats[bi]
                    # vertical 3-tap via banded matmuls. Each field's PSUM
                    # tile spans ceil(W/512) banks; matmuls fill 512-wide
                    # slices, post-pool ops then read the full W at once.
                    ps = {}
                    for name in ("p", "t", "u2", "v2"):
                        pt = psumpool.tile([128, W], f32, tag=f"ps{name}",
                                           name=f"ps{name}")
                        for ci in range(n_chunks):
                            c0 = ci * MAXW_PSUM
                            cw = min(MAXW_PSUM, W - c0)
                            nc.tensor.matmul(
                                pt[0:n_out, c0:c0 + cw],
                                lhsT=bmat[0:nr, 0:n_out],
                                rhs=hs[name][0:nr, c0:c0 + cw],
                                start=True, stop=True,
                            )
                        ps[name] = pt
                    X, Y = ps["p"], ps["t"]
                    G, Hh = ps["u2"], ps["v2"]
                    ro = slice(0, n_out)
                    co = slice(0, W)

                    def pt_(tag):
                        return ppool.tile([128, W], f32, tag=tag, name=tag)

                    # stage Y, Hh to SBUF (one PSUM operand max per op)
                    Ysb = pt_("Ysb")
                    nc.scalar.copy(Ysb[ro, co], Y[ro, co])
                    Hsb = pt_("Hsb")
                    nc.scalar.copy(Hsb[ro, co], Hh[ro, co])
                    A2 = pt_("A2")   # 2*mu_x*mu_y
                    nc.vector.scalar_tensor_tensor(
                        A2[ro, co], X[ro, co], 2.0, Ysb[ro, co],
                        op0=Alu.mult, op1=Alu.mult)
                    Dd = pt_("Dd")   # G - H
                    nc.vector.tensor_sub(Dd[ro, co], G[ro, co], Hsb[ro, co])
                    M = pt_("M")     # G + H
                    nc.vector.tensor_add(M[ro, co], G[ro, co], Hsb[ro, co])
                    V = pt_("V")     # mu_x^2 + mu_y^2
                    nc.vector._custom_dve(_OP_SQSUM, out=V[ro, co],
                                          in0=X[ro, co], in1=Ysb[ro, co])
                    n1n2 = pt_("n1n2")
                    nc.vector._custom_dve(
                        _OP_SSIM_RAT, out=n1n2[ro, co], in0=A2[ro, co],
                        in1=Dd[ro, co], s0=float(C1), s1=0.5, imm2=float(C2))
                    d1d2 = pt_("d1d2")
                    nc.vector._custom_dve(
                        _OP_SSIM_RAT, out=d1d2[ro, co], in0=V[ro, co],
                        in1=M[ro, co], s0=float(C1), s1=0.5, imm2=float(C2))
                    rcp = pt_("rcp")
                    nc.vector.reciprocal_approx_fast(rcp[ro, co], d1d2[ro, co])
                    fin = pt_("fin")
                    spart = ppool.tile([128, 1], f32, tag="spart")
                    nc.vector._custom_dve(
                        _OP_SSIM_FINAL, out=fin[ro, co], in0=n1n2[ro, co],
                        in1=rcp[ro, co], s0=1.0, s1=-1.0, imm2=0.5,
                        accum_out=spart[ro, :])
                    nc.vector.tensor_add(
                        acc[0:n_out, 0:1], acc[0:n_out, 0:1], spart[ro, :]
                    )
                    nc.vector.tensor_add(
                        acc[0:k_l1, 1:2], acc[0:k_l1, 1:2], l1part[0:k_l1, :]
                    )

            nc.sync.dma_start(out=acc_d[:, :], in_=acc[:, :])

    nc.compile()
    return nc


_CACHE = {}


def _get_program(n_img, H, W):
    key = (n_img, H, W)
    if key not in _CACHE:
        _CACHE[key] = build_program(n_img, H, W)
    return _CACHE[key]


def _pad_cols(x):
    """Add one zero column on each side of the last dim."""
    r, w = x.shape
    out = np.zeros((r, w + 2), dtype=np.float32)
    out[:, 1:w + 1] = x
    return out


LAST_RESULTS = None


def kernel(pred, target):
    import os

    from concourse.bass_utils import run_bass_kernel_spmd

    global LAST_RESULTS

    pred = np.asarray(pred, dtype=np.float32).reshape(16, IMG_H, IMG_W)
    target = np.asarray(target, dtype=np.float32).reshape(16, IMG_H, IMG_W)

    nc = _get_program(N_IMG_PER_CORE, IMG_H, IMG_W)
    bm = make_bmats(IMG_H)

    in_maps = []
    for c in range(N_CORES):
        sl = slice(c * N_IMG_PER_CORE, (c + 1) * N_IMG_PER_CORE)
        p = pred[sl].reshape(N_IMG_PER_CORE * IMG_H, IMG_W)
        t = target[sl].reshape(N_IMG_PER_CORE * IMG_H, IMG_W)
        in_maps.append({
            "pred": _pad_cols(p),
            "target": _pad_cols(t),
            "bmats": bm,
        })

    trace = bool(int(os.environ.get("KERNEL_TRACE", "0")))
    res = run_bass_kernel_spmd(nc, in_maps, list(range(N_CORES)), trace=trace)
    LAST_RESULTS = res
    ssim_sum = 0.0
    l1_sum = 0.0
    for r in res.results:
        acc = r["acc_out"]
        ssim_sum += float(acc[:, 0].sum(dtype=np.float64))
        l1_sum += float(acc[:, 1].sum(dtype=np.float64))
    n = 16.0 * IMG_H * IMG_W
    loss = ALPHA * (ssim_sum / n) + BETA * (l1_sum / n)
    return np.float32(loss)
